# revision 2
# baseline (speedup 1.0000x reference)
"""nn_BlockLowRank kernel for 8 Trainium2 NeuronCores.

Math: W[io, jn] = sum_r core1[i,o,j,r] * core2[i,r,j,n]  (io=i*64+o, jn=j*64+n)
      out = bias + X @ W.T,  X = x.reshape(16384, 4096)

Strategy (data-parallel over batch, no collectives):
  - Each core takes 2048 rows of X. Host uploads X_shard.T (layout only).
  - Host rearranges core1/core2 (fp16, zero-padded block-diagonal) so the
    device can build W.T tiles with K=32 matmuls (4-way row-packed).
  - Phase A (device): build all W.T tiles -> DRAM scratch (f32).
  - Phase B (device): out.T[io,b] = sum_jn W.T[jn,io].T @ X.T[jn,b] in f32r
    (full-rate fp32-storage matmul, ~1e-3 rel err), bias fused into the
    ScalarE PSUM->SBUF evacuation.
  - Host transposes out.T shards back and concatenates.
"""

import os
import sys
import types

sys.path.insert(0, "/opt/trn_rl_repo")

import numpy as np


def _install_axon_hooks():
    """Provide antenv.axon_hooks (absent in this image) so trace=True works."""
    import antenv

    if "antenv.axon_hooks" in sys.modules:
        return
    mod = types.ModuleType("antenv.axon_hooks")
    store = {}
    mod.set_axon_ntff_profile_hook = lambda h: store.__setitem__("h", h)
    mod.get_axon_ntff_profile_hook = lambda: store.get("h")
    sys.modules["antenv.axon_hooks"] = mod
    antenv.axon_hooks = mod
    try:
        from trn_agent_boot.trn_boot import _ntff_profile_via_ctypes

        mod.set_axon_ntff_profile_hook(
            _ntff_profile_via_ctypes("/opt/axon/libaxon_pjrt.so")
        )
    except Exception:
        pass


_install_axon_hooks()

import concourse.bass as bass  # noqa: E402
import concourse.mybir as mybir  # noqa: E402
import concourse.tile as tile  # noqa: E402
from concourse import bacc  # noqa: E402
from concourse.bass_utils import run_bass_kernel_spmd  # noqa: E402

F32 = mybir.dt.float32
F32R = mybir.dt.float32r
F16 = mybir.dt.float16

N_CORES = 8
D = 4096          # in_dim == out_dim
B_TOTAL = 8 * 2048
BL = B_TOTAL // N_CORES   # 2048 batch rows per core
N_T = D // 128            # 32 io-tiles
N_JNC = D // 128          # 32 jn-chunks
N_TG = 8                  # tau-groups per io-tile (4 jnc each)
BCH = 1024                # X.T chunk resident in SBUF
BSUB = 512                # matmul moving width (f32r full-rate needs >=256)

_PROG_CACHE = {}


def _build_program():
    nc = bacc.Bacc("TRN2", target_bir_lowering=False, debug=False,
                   num_devices=N_CORES)

    x_t = nc.dram_tensor("x_t", [D, BL], F32R, kind="ExternalInput")
    c2s = nc.dram_tensor("c2s", [64, N_TG, 128, 128], F16, kind="ExternalInput")
    c1s = nc.dram_tensor("c1s", [64, N_TG, 128, 64], F16, kind="ExternalInput")
    bias_t = nc.dram_tensor("bias_t", [128, N_T], F32, kind="ExternalInput")
    out_t = nc.dram_tensor("out_t", [D, BL], F32, kind="ExternalOutput")
    wt_scr = nc.dram_tensor("wt_scr", [N_T, 128, D], F32)  # W.T tiles scratch

    with tile.TileContext(nc) as tc:
        with tc.tile_pool(name="cc", bufs=6) as cc_pool, \
             tc.tile_pool(name="wstage", bufs=3) as st_pool, \
             tc.tile_pool(name="xt", bufs=1) as xt_pool, \
             tc.tile_pool(name="wt", bufs=2) as wt_pool, \
             tc.tile_pool(name="outp", bufs=3) as out_pool, \
             tc.tile_pool(name="misc", bufs=1) as misc_pool, \
             tc.tile_pool(name="wps", bufs=6, space="PSUM") as wps_pool, \
             tc.tile_pool(name="mps", bufs=2, space="PSUM") as mps_pool:

            # ---------------- Phase A: build W.T tiles ----------------
            for t in range(N_T):
                i0, i1 = 2 * t, 2 * t + 1
                for tg in range(N_TG):
                    c2a = cc_pool.tile([128, 128], F16, tag="c2")
                    c2b = cc_pool.tile([128, 128], F16, tag="c2")
                    c1a = cc_pool.tile([128, 64], F16, tag="c1")
                    c1b = cc_pool.tile([128, 64], F16, tag="c1")
                    nc.sync.dma_start(out=c2a[:], in_=c2s[i0, tg])
                    nc.sync.dma_start(out=c2b[:], in_=c2s[i1, tg])
                    nc.sync.dma_start(out=c1a[:], in_=c1s[i0, tg])
                    nc.sync.dma_start(out=c1b[:], in_=c1s[i1, tg])
                    stage = st_pool.tile([128, 512], F32, tag="st")
                    for g in range(4):
                        jnc = 4 * tg + g
                        ps = wps_pool.tile([128, 128], F32, tag="wb")
                        r0, r1 = 32 * g, 32 * (g + 1)
                        nc.tensor.matmul(ps[:, 0:64], c2a[r0:r1, :],
                                         c1a[r0:r1, :], start=True, stop=True,
                                         tile_position=(r0, 0))
                        nc.tensor.matmul(ps[:, 64:128], c2b[r0:r1, :],
                                         c1b[r0:r1, :], start=True, stop=True,
                                         tile_position=(r0, 0))
                        nc.scalar.copy(stage[:, 128 * g:128 * (g + 1)], ps[:])
                    nc.sync.dma_start(
                        out=wt_scr[t, :, 512 * tg:512 * (tg + 1)], in_=stage[:])

            # ---------------- Phase B: main matmul ----------------
            bias_sb = misc_pool.tile([128, N_T], F32)
            nc.sync.dma_start(out=bias_sb[:], in_=bias_t[:])

            n_bch = BL // BCH
            n_sub = BCH // BSUB
            for bc in range(n_bch):
                xts = []
                for jnc in range(N_JNC):
                    xt_tile = xt_pool.tile([128, BCH], F32R, tag=f"x{jnc}")
                    nc.sync.dma_start(
                        out=xt_tile[:],
                        in_=x_t[128 * jnc:128 * (jnc + 1),
                                BCH * bc:BCH * (bc + 1)])
                    xts.append(xt_tile)
                for t in range(N_T):
                    wt_sb = wt_pool.tile([128, D], F32R, tag="w")
                    nc.gpsimd.dma_start(out=wt_sb[:], in_=wt_scr[t])
                    for sub in range(n_sub):
                        ps = mps_pool.tile([128, BSUB], F32, tag="m")
                        c0 = BSUB * sub
                        for jnc in range(N_JNC):
                            nc.tensor.matmul(
                                ps[:],
                                wt_sb[:, 128 * jnc:128 * (jnc + 1)],
                                xts[jnc][:, c0:c0 + BSUB],
                                start=(jnc == 0), stop=(jnc == N_JNC - 1))
                        oc = out_pool.tile([128, BSUB], F32, tag="o")
                        nc.scalar.activation(
                            oc[:], ps[:], mybir.ActivationFunctionType.Identity,
                            bias=bias_sb[:, t:t + 1])
                        nc.sync.dma_start(
                            out=out_t[128 * t:128 * (t + 1),
                                      BCH * bc + c0:BCH * bc + c0 + BSUB],
                            in_=oc[:])

    nc.compile()
    return nc


def _host_prep(x, core1, core2, bias):
    x = np.ascontiguousarray(x, dtype=np.float32).reshape(B_TOTAL, D)
    core1 = np.asarray(core1, dtype=np.float32)
    core2 = np.asarray(core2, dtype=np.float32)
    bias = np.asarray(bias, dtype=np.float32)

    # c2pad[i, jnc, (j_l,r)=32, (j_l,n)=128]: block-diagonal over j_l
    c2r = core2.reshape(64, 16, N_JNC, 2, 64)          # i, r, jnc, j_l, n
    c2pad = np.zeros((64, N_JNC, 2, 16, 128), dtype=np.float16)
    c2pad[:, :, 0, :, 0:64] = c2r[:, :, :, 0, :].transpose(0, 2, 1, 3)
    c2pad[:, :, 1, :, 64:128] = c2r[:, :, :, 1, :].transpose(0, 2, 1, 3)
    # stack 4 jnc per tau-group along rows -> [i, tg, 128, 128]
    c2s = c2pad.reshape(64, N_JNC, 32, 128).reshape(64, N_TG, 128, 128)

    # c1r[i, jnc, (j_l,r)=32, o=64] = core1[i, o, 2*jnc+j_l, r]
    c1j = core1.transpose(0, 2, 3, 1)                  # i, j, r, o
    c1r = c1j.reshape(64, N_JNC, 2, 16, 64).reshape(64, N_JNC, 32, 64)
    c1s = np.ascontiguousarray(c1r.reshape(64, N_TG, 128, 64),
                               dtype=np.float16)

    bias_t = np.ascontiguousarray(bias.reshape(N_T, 128).T, dtype=np.float32)

    in_maps = []
    for c in range(N_CORES):
        xs = x[c * BL:(c + 1) * BL, :]
        x_t = np.ascontiguousarray(xs.T)
        in_maps.append({"x_t": x_t, "c2s": np.ascontiguousarray(c2s),
                        "c1s": c1s, "bias_t": bias_t})
    return in_maps


def kernel(x, core1, core2, bias):
    if "prog" not in _PROG_CACHE:
        _PROG_CACHE["prog"] = _build_program()
    nc = _PROG_CACHE["prog"]

    in_maps = _host_prep(x, core1, core2, bias)
    trace = bool(int(os.environ.get("KERNEL_TRACE", "0")))
    res = run_bass_kernel_spmd(nc, in_maps, core_ids=list(range(N_CORES)),
                               trace=trace)
    if trace:
        _PROG_CACHE["last_result"] = res

    outs = []
    for c in range(N_CORES):
        outs.append(res.results[c]["out_t"].T)   # [BL, D]
    out = np.concatenate(outs, axis=0)           # [B_TOTAL, D]
    return np.ascontiguousarray(out.reshape(8, 2048, D), dtype=np.float32)
